# revision 2
# baseline (speedup 1.0000x reference)
"""GaussianNB log-posterior kernel for 8 Trainium2 NeuronCores.

out[b, c] = log_pi[c] - 0.5 * sum_f(log2pi + log_var[c,f] + (x[b,f]-mu[c,f])^2 / var[c,f])

Strategy: data-parallel over the batch dim (B=2048 -> 256 rows/core).
mu/log_var/log_pi replicated. Per core:
  - prep (natural layout): inv = exp(-lv); wc = mu*inv (f32r); wq = -0.5*inv (f32r);
    const_c = log_pi - 0.5*(F*log2pi + sum_f lv + sum_f mu^2*inv)
  - PE transposes: x (fp32) and wq/wc (f32r) to f-major layout; squares of xT on ACT
  - GEMM (f32r, full-rate at N>=256): outT[c,b] = sum_k wqT*x2T + wcT*xT, + const epilogue
Output per core is (C, 256) = transposed slice; host reassembles.
"""
import sys

sys.path.insert(0, "/opt/trn_rl_repo")
import numpy as np
import concourse.bacc as bacc
import concourse.mybir as mybir
from concourse.tile import TileContext
from concourse.bass_utils import run_bass_kernel_spmd
from concourse.masks import make_identity

B, C, F = 2048, 256, 1024
NCORES = 8
BSH = B // NCORES  # 256
KT = F // 128      # 8 k-tiles
LOG_2PI = float(np.log(2.0 * np.pi))
F32 = mybir.dt.float32
F32R = mybir.dt.float32r
AX = mybir.AxisListType.X
OP = mybir.AluOpType
AF = mybir.ActivationFunctionType

_CACHE = {}


def _build():
    nc = bacc.Bacc("TRN2", target_bir_lowering=False, debug=False, num_devices=NCORES)
    x_d = nc.dram_tensor("x", [BSH, F], F32, kind="ExternalInput").ap()
    mu_d = nc.dram_tensor("mu", [C, F], F32, kind="ExternalInput").ap()
    lv_d = nc.dram_tensor("lv", [C, F], F32, kind="ExternalInput").ap()
    lp_d = nc.dram_tensor("lp", [C, 1], F32, kind="ExternalInput").ap()
    out_d = nc.dram_tensor("out", [C, BSH], F32, kind="ExternalOutput").ap()

    with TileContext(nc) as tc:
        with (
            tc.tile_pool(name="sb", bufs=1) as sb,
            tc.tile_pool(name="tp", bufs=2, space="PSUM") as tp,
            tc.tile_pool(name="po", bufs=1, space="PSUM") as po,
        ):
            # ---------- DMA in ----------
            x_nat = [sb.tile([128, F], F32, tag=f"x{m}", name=f"x{m}") for m in range(2)]
            mu_nat = [sb.tile([128, F], F32, tag=f"mu{m}", name=f"mu{m}") for m in range(2)]
            lv_nat = [sb.tile([128, F], F32, tag=f"lv{m}", name=f"lv{m}") for m in range(2)]
            lp = [sb.tile([128, 1], F32, tag=f"lp{m}", name=f"lp{m}") for m in range(2)]
            for m in range(2):
                nc.sync.dma_start(out=x_nat[m][:], in_=x_d[m * 128:(m + 1) * 128, :])
            for m in range(2):
                nc.sync.dma_start(out=mu_nat[m][:], in_=mu_d[m * 128:(m + 1) * 128, :])
                nc.sync.dma_start(out=lv_nat[m][:], in_=lv_d[m * 128:(m + 1) * 128, :])
                nc.sync.dma_start(out=lp[m][:], in_=lp_d[m * 128:(m + 1) * 128, :])

            ident = sb.tile([128, 128], F32, tag="id")
            make_identity(nc, ident[:])
            identr = sb.tile([128, 128], F32R, tag="idr")
            nc.vector.tensor_copy(identr[:], ident[:])

            # ---------- x transposes (fp32, exact) + f32r rounding on copyback ----------
            xT = sb.tile([128, KT, BSH], F32R, tag="xT")
            x2T = sb.tile([128, KT, BSH], F32R, tag="x2T")
            for kq in range(KT // 4):  # quads of k-tiles -> (128,1024) psum (2 banks)
                p = tp.tile([128, 1024], F32, tag="tp")
                for j in range(4):
                    k = 4 * kq + j
                    for m in range(2):
                        nc.tensor.transpose(
                            p[:, j * 256 + m * 128: j * 256 + m * 128 + 128],
                            x_nat[m][:, k * 128:(k + 1) * 128],
                            ident[:],
                        )
                nc.vector.tensor_copy(xT[:, 4 * kq:4 * kq + 4, :], p[:])
                nc.scalar.activation(x2T[:, 4 * kq:4 * kq + 4, :], p[:], AF.Square)

            # ---------- W prep ----------
            inv = [sb.tile([128, F], F32, tag=f"inv{m}", name=f"inv{m}") for m in range(2)]
            wc_nat = [sb.tile([128, F], F32R, tag=f"wc{m}", name=f"wc{m}") for m in range(2)]
            wq_nat = [sb.tile([128, F], F32R, tag=f"wq{m}", name=f"wq{m}") for m in range(2)]
            m2i = [sb.tile([128, F], F32, tag=f"m2i{m}", name=f"m2i{m}") for m in range(2)]
            const = [sb.tile([128, 1], F32, tag=f"c{m}", name=f"c{m}") for m in range(2)]
            for m in range(2):
                nc.scalar.activation(inv[m][:], lv_nat[m][:], AF.Exp, scale=-1.0)
                nc.vector.tensor_mul(wc_nat[m][:], mu_nat[m][:], inv[m][:])
                nc.vector.tensor_scalar_mul(wq_nat[m][:], inv[m][:], -0.5)
                nc.gpsimd.tensor_mul(m2i[m][:], mu_nat[m][:], wc_nat[m][:].bitcast(F32))
                slv = sb.tile([128, 1], F32, tag=f"slv{m}")
                sm2i = sb.tile([128, 1], F32, tag=f"sm2i{m}")
                nc.vector.reduce_sum(slv[:], lv_nat[m][:], axis=AX)
                nc.vector.reduce_sum(sm2i[:], m2i[m][:], axis=AX)
                t = sb.tile([128, 1], F32, tag=f"t{m}")
                nc.vector.tensor_add(t[:], slv[:], sm2i[:])
                t2 = sb.tile([128, 1], F32, tag=f"t2{m}")
                nc.vector.tensor_scalar(t2[:], t[:], -0.5, -0.5 * F * LOG_2PI, OP.mult, OP.add)
                nc.vector.tensor_add(const[m][:], t2[:], lp[m][:])

            # ---------- W transposes (f32r transpose mode) ----------
            wqT = sb.tile([128, KT, C], F32R, tag="wqT")
            wcT = sb.tile([128, KT, C], F32R, tag="wcT")
            cb = 0  # copyback engine alternation
            for nat, T in ((wq_nat, wqT), (wc_nat, wcT)):
                for kq in range(KT // 4):
                    p = tp.tile([128, 1024], F32R, tag="tp")
                    for j in range(4):
                        k = 4 * kq + j
                        for m in range(2):
                            nc.tensor.transpose(
                                p[:, j * 256 + m * 128: j * 256 + m * 128 + 128],
                                nat[m][:, k * 128:(k + 1) * 128],
                                identr[:],
                            )
                    if cb != 3:
                        nc.scalar.copy(out=T[:, 4 * kq:4 * kq + 4, :], in_=p[:])
                    else:
                        nc.vector.tensor_copy(T[:, 4 * kq:4 * kq + 4, :], p[:])
                    cb += 1

            # ---------- GEMM + epilogue ----------
            for m in range(2):
                pg = po.tile([128, BSH], F32, tag=f"pg{m}")
                step = 0
                for T, A in ((wqT, x2T), (wcT, xT)):
                    for k in range(KT):
                        nc.tensor.matmul(
                            pg[:],
                            T[:, k, m * 128:(m + 1) * 128],
                            A[:, k, :],
                            start=(step == 0),
                            stop=(step == 2 * KT - 1),
                        )
                        step += 1
                out_sb = sb.tile([128, BSH], F32, tag=f"os{m}")
                nc.vector.tensor_scalar_add(out_sb[:], pg[:], const[m][:])
                nc.sync.dma_start(out=out_d[m * 128:(m + 1) * 128, :], in_=out_sb[:])

    nc.compile()
    return nc


def get_nc():
    if "nc" not in _CACHE:
        _CACHE["nc"] = _build()
    return _CACHE["nc"]


def make_in_maps(inputs):
    x = np.ascontiguousarray(np.asarray(inputs["x"], dtype=np.float32))
    mu = np.ascontiguousarray(np.asarray(inputs["mu"], dtype=np.float32))
    lv = np.ascontiguousarray(np.asarray(inputs["log_var"], dtype=np.float32))
    lp = np.ascontiguousarray(
        np.asarray(inputs["log_pi"], dtype=np.float32)).reshape(C, 1)
    return [
        {"x": np.ascontiguousarray(x[c * BSH:(c + 1) * BSH]),
         "mu": mu, "lv": lv, "lp": lp}
        for c in range(NCORES)
    ]


def kernel(x, mu, log_var, log_pi):
    nc = get_nc()
    in_maps = make_in_maps(
        {"x": x, "mu": mu, "log_var": log_var, "log_pi": log_pi})
    res = run_bass_kernel_spmd(nc, in_maps, list(range(NCORES)))
    out = np.empty((B, C), dtype=np.float32)
    for c in range(NCORES):
        out[c * BSH:(c + 1) * BSH, :] = res.results[c]["out"].T
    return out

